# revision 2
# baseline (speedup 1.0000x reference)
"""Trainium2 Bass kernel v4 for nn_DNN_sym_10101763080772 (moe_routing).

All-int8 weight streaming. v3 -> v4 (HW-measured):
  - DVE int8->bf16 CAST measured 0.55 ns/col (2x mode), ACT 0.87 ns/col:
    combined ~380 G elem/s exceeds the int8 DMA arrival rate (~313 G/s),
    so alpha goes to 1: EVERY mtile ships int8. No bf16 slab at all.
  - Convert split rebalanced: ACT 39% / DVE 61% of each chunk.
  - Chunk sizes [1,2,2,2,1] mtiles per layer: small first chunk starts the
    convert pipeline earlier, small last chunk shrinks the end tail.
  - Chunks alternate sync/scalar HWDGE rings to close inter-transfer gaps.

Math per core (row-shard L1 / col-shard L2, partial q summed on host):
  h1' = Q1 @ h0 + bl1/s1          all rows s1 = rowmax/127, Q1 int8
  p2  = Q2' @ h1'                 Q2' = rowquant(W2 * s1[col]) int8
  q   = (Wo*s2')[:,m] @ p2 (+ bo + Wo@bl2 on core 0)
"""

import os
import sys

import numpy as np

if "/opt/trn_rl_repo" not in sys.path:
    sys.path.insert(0, "/opt/trn_rl_repo")

N_CORES = 8
NA = 128
D = 8192
SH = D // N_CORES  # 1024

# fp32-column offsets inside the packed constant blob (bitcast from int8)
_C_X = 0          # [*, 0:3] x
_C_ONES = 3
_C_BL0 = 4        # 64
_C_WL0 = 68       # 576
_C_BL1 = 644      # 8   bl1/s1, partition-major
_C_WOT = 652      # 192 (Wo * s2') tiled
_C_BO = 844       # 1   bo + Wo@bl2 (core 0)
_C_ONESROW = 845  # 128 (partition 0)
_C_X4 = 973       # 134 xTa | W1aug | W12aug on partitions 0:4
_C_ATF = 1107     # 1   atom_list as float
_CW = 1108        # fp32 cols
_CB = 4 * _CW     # int8 cols

CH = 16384             # stream tile width (2 mtiles)
CHUNKS = [1, 2, 2, 2, 1]  # mtiles per chunk, per layer
A_FULL = 6336          # ACT cols of a full chunk (DVE takes the rest)

_session = {}


def _build():
    import concourse.bass as bass
    import concourse.mybir as mybir
    import concourse.tile as tile
    from concourse import bacc

    f32 = mybir.dt.float32
    i8 = mybir.dt.int8
    bf16 = mybir.dt.bfloat16

    nc = bacc.Bacc("TRN2", target_bir_lowering=False, debug=False)

    i8w = _CB + 16 * 8192
    slab8_d = nc.dram_tensor("slab8", [128, i8w], i8, kind="ExternalInput")
    q_d = nc.dram_tensor("q", [3, 1], f32, kind="ExternalOutput")

    add = mybir.AluOpType.add
    sub = mybir.AluOpType.subtract
    mult = mybir.AluOpType.mult
    is_eq = mybir.AluOpType.is_equal

    with tile.TileContext(nc) as tc:
        with (
            tc.tile_pool(name="const", bufs=1) as cp,
            tc.tile_pool(name="work", bufs=1) as wk,
            tc.tile_pool(name="s8", bufs=4) as s8,
            tc.tile_pool(name="cv", bufs=3) as cv,
            tc.tile_pool(name="ps", bufs=1, space=bass.MemorySpace.PSUM) as pp,
        ):
            # ---- constants ride the front of the int8 slab (sync ring) ----
            cb = cp.tile([128, _CB], i8)
            nc.sync.dma_start(out=cb[:], in_=slab8_d[:, 0:_CB])
            cbf = cb[:].bitcast(f32)  # [128, _CW]

            x_sb = cbf[:, _C_X : _C_X + 3]
            ones = cbf[:, _C_ONES : _C_ONES + 1]
            bl0p = cbf[:, _C_BL0 : _C_BL0 + 64]
            bl1p = cbf[:, _C_BL1 : _C_BL1 + 8]
            wot = cbf[:, _C_WOT : _C_WOT + 192]
            bo = cbf[0:3, _C_BO : _C_BO + 1]
            ones_row = cbf[0:1, _C_ONESROW : _C_ONESROW + 128]
            xTa = cbf[0:4, _C_X4 : _C_X4 + 128]
            w1aug = cbf[0:4, _C_X4 + 128 : _C_X4 + 131]
            w12aug = cbf[0:4, _C_X4 + 131 : _C_X4 + 134]
            atomf = cbf[:, _C_ATF : _C_ATF + 1]

            # ---- issue every weight DMA up front, alternating rings ----
            l1_chunks, l2_chunks = [], []
            off = _CB
            ci = 0
            for chunks in (l1_chunks, l2_chunks):
                for nmt in CHUNKS:
                    t = s8.tile([128, CH], i8, tag="w8")
                    w = nmt * 8192
                    eng = nc.sync if ci % 2 == 0 else nc.scalar
                    eng.dma_start(out=t[:, 0:w], in_=slab8_d[:, off : off + w])
                    chunks.append((t, nmt))
                    off += w
                    ci += 1

            # ---- routed embedding: g = select(atom==1, g1, g12) ----
            g1p = pp.tile([NA, 3], f32)
            g12p = pp.tile([NA, 3], f32)
            nc.tensor.matmul(g1p[:], xTa, w1aug, start=True, stop=True)
            nc.tensor.matmul(g12p[:], xTa, w12aug, start=True, stop=True)

            mask = wk.tile([NA, 1], f32)
            nc.vector.tensor_single_scalar(mask[:], atomf, 1, is_eq)
            g12_sb = wk.tile([NA, 3], f32)
            nc.vector.tensor_copy(g12_sb[:], g12p[:])
            diff = wk.tile([NA, 3], f32)
            nc.vector.tensor_tensor(diff[:], g1p[:], g12_sb[:], sub)
            g_sb = wk.tile([NA, 3], f32)
            nc.vector.scalar_tensor_tensor(g_sb[:], diff[:], mask[:], g12_sb[:], mult, add)

            # ---- d = vec(g.T @ x), broadcast to all partitions ----
            gx = wk.tile([NA, 9], f32)
            for a in range(3):
                nc.vector.tensor_scalar_mul(
                    gx[:, 3 * a : 3 * a + 3], x_sb, g_sb[:, a : a + 1]
                )
            drp = pp.tile([1, 9], f32)
            nc.tensor.matmul(drp[:], ones, gx[:], start=True, stop=True)
            drow = wk.tile([1, 9], f32)
            nc.vector.tensor_copy(drow[:], drp[:])
            dbp = pp.tile([128, 9], f32)
            nc.tensor.matmul(dbp[:], ones_row, drow[:], start=True, stop=True)
            dbc = wk.tile([128, 9], f32)
            nc.vector.tensor_copy(dbc[:], dbp[:])

            # ---- h0 = Wl0 @ d + bl0 on the Vector engine, [128, 64] ----
            acc_a = wk.tile([128, 64], f32)
            acc_b = wk.tile([128, 64], f32)
            h0 = wk.tile([128, 64], bf16)
            cur, nxt = acc_a, acc_b
            nc.vector.scalar_tensor_tensor(
                cur[:], cbf[:, _C_WL0 : _C_WL0 + 64], dbc[:, 0:1], bl0p, mult, add
            )
            for k in range(1, 9):
                dst = h0 if k == 8 else nxt
                nc.vector.scalar_tensor_tensor(
                    dst[:],
                    cbf[:, _C_WL0 + 64 * k : _C_WL0 + 64 * (k + 1)],
                    dbc[:, k : k + 1],
                    cur[:],
                    mult,
                    add,
                )
                cur, nxt = nxt, cur

            def convert(t8, nmt):
                """int8 -> bf16 on ACT + DVE; returns the bf16 tile."""
                t = cv.tile([128, CH], bf16, tag="cvt")
                w = nmt * 8192
                a = A_FULL * nmt // 2
                nc.scalar.copy(out=t[:, 0:a], in_=t8[:, 0:a])
                nc.vector.tensor_copy(t[:, a:w], t8[:, a:w])
                return t

            # ---- layer 1: h1' = Q1 @ h0 + bl1/s1, rows sharded ----
            h1p = pp.tile([128, 8], f32)
            mt = 0
            for t8, nmt in l1_chunks:
                wt = convert(t8, nmt)
                for j in range(nmt):
                    for kt in range(64):
                        nc.tensor.matmul(
                            h1p[:, mt : mt + 1],
                            wt[:, j * 8192 + kt * 128 : j * 8192 + (kt + 1) * 128],
                            h0[:, kt : kt + 1],
                            start=(kt == 0),
                            stop=(kt == 63),
                        )
                    mt += 1
            h1 = wk.tile([128, 8], bf16)
            nc.vector.tensor_tensor(h1[:], h1p[:], bl1p, add)

            # ---- layer 2 + q contraction in 2 halves ----
            p2p = pp.tile([128, 64], f32)
            p2sb = wk.tile([128, 64], f32)
            qp = pp.tile([3, 1], f32)

            def q_range(c0, c1):
                nc.vector.tensor_copy(p2sb[:, c0:c1], p2p[:, c0:c1])
                for ch in range(c0, c1):
                    nc.tensor.matmul(
                        qp[:],
                        wot[:, ch * 3 : (ch + 1) * 3],
                        p2sb[:, ch : ch + 1],
                        start=(ch == 0),
                        stop=(ch == 63),
                    )

            g2 = 0
            for t8, nmt in l2_chunks:
                wt = convert(t8, nmt)
                for j in range(nmt):
                    for mtl in range(8):
                        mt2 = 8 * g2 + mtl
                        for kc in range(8):
                            o = j * 8192 + mtl * 1024 + kc * 128
                            nc.tensor.matmul(
                                p2p[:, mt2 : mt2 + 1],
                                wt[:, o : o + 128],
                                h1[:, kc : kc + 1],
                                start=(kc == 0),
                                stop=(kc == 7),
                            )
                    g2 += 1
                if g2 == 5:
                    q_range(0, 32)
            q_range(32, 64)

            q_sb = wk.tile([3, 1], f32)
            nc.vector.tensor_tensor(q_sb[:], qp[:], bo, add)
            nc.sync.dma_start(out=q_d[:], in_=q_sb[:])

    nc.compile()
    return nc


def _quant_rows(W):
    s = np.abs(W).max(axis=1, keepdims=True) / 127.0
    s[s == 0] = 1.0
    q = np.rint(W / s).astype(np.int8)
    return q, s[:, 0].astype(np.float32)


def _prep_in_maps(inputs):
    f = lambda k: np.asarray(inputs[k], np.float32)
    x = f("x")
    W1, b1, W12, b12 = f("W1"), f("b1"), f("W12"), f("b12")
    Wl0, bl0 = f("Wl0"), f("bl0")
    Wl1, bl1 = f("Wl1"), f("bl1")
    Wl2, bl2 = f("Wl2"), f("bl2")
    Wo, bo = f("Wo"), f("bo")
    atom = np.asarray(inputs["atom_list"], np.float32)

    # per-core L1 row scales; every row int8-quantized
    s1_global = np.empty(D, np.float32)
    q1_cores = []
    for i in range(N_CORES):
        rows = slice(SH * i, SH * (i + 1))
        q1, s1 = _quant_rows(Wl1[rows])
        q1_cores.append(q1)
        s1_global[rows] = s1

    # W2' = W2 * s1[col] (uniform column scales); per-row int8 quantization
    W2p = Wl2 * s1_global[None, :]
    q2_all, s2_full = _quant_rows(W2p)
    Wo_s = Wo * s2_full[None, :]
    bo_eff = bo + Wo @ bl2

    blob = np.zeros((128, _CW), np.float32)
    blob[:, _C_X : _C_X + 3] = x
    blob[:, _C_ONES] = 1.0
    blob[:, _C_BL0 : _C_BL0 + 64] = bl0.reshape(64, 128).T
    blob[:, _C_WL0 : _C_WL0 + 576] = (
        Wl0.reshape(64, 128, 9).transpose(1, 2, 0).reshape(128, 576)
    )
    blob[:, _C_WOT : _C_WOT + 192] = (
        Wo_s.reshape(3, 64, 128).transpose(2, 1, 0).reshape(128, 192)
    )
    blob[0:3, _C_BO] = bo_eff
    blob[0, _C_ONESROW : _C_ONESROW + 128] = 1.0
    blob[0:3, _C_X4 : _C_X4 + 128] = x.T
    blob[3, _C_X4 : _C_X4 + 128] = 1.0
    blob[0:3, _C_X4 + 128 : _C_X4 + 131] = W1.T
    blob[3, _C_X4 + 128 : _C_X4 + 131] = b1
    blob[0:3, _C_X4 + 131 : _C_X4 + 134] = W12.T
    blob[3, _C_X4 + 131 : _C_X4 + 134] = b12
    blob[:, _C_ATF] = atom

    in_maps = []
    for i in range(N_CORES):
        rows = slice(SH * i, SH * (i + 1))
        b = blob.copy()
        bl1_eff = bl1[rows] / s1_global[rows]
        b[:, _C_BL1 : _C_BL1 + 8] = bl1_eff.reshape(8, 128).T
        if i != 0:
            b[0:3, _C_BO] = 0.0

        l1_i8 = np.ascontiguousarray(
            q1_cores[i].reshape(8, 128, 64, 128).transpose(3, 0, 2, 1).reshape(128, 65536)
        )
        l2_i8 = np.ascontiguousarray(
            q2_all[:, rows].reshape(64, 128, 8, 128).transpose(3, 0, 2, 1).reshape(128, 65536)
        )
        slab8 = np.concatenate(
            [b.view(np.int8).reshape(128, _CB), l1_i8, l2_i8], axis=1
        )
        in_maps.append({"slab8": slab8})
    return in_maps


def _install_profile_shim():
    import types

    try:
        from antenv.axon_hooks import get_axon_ntff_profile_hook  # noqa: F401
        return
    except ImportError:
        pass
    try:
        import antenv
        from trn_agent_boot.trn_boot import _ntff_profile_via_ctypes

        mod = types.ModuleType("antenv.axon_hooks")
        holder = {"h": None}
        mod.set_axon_ntff_profile_hook = lambda h: holder.__setitem__("h", h)
        mod.get_axon_ntff_profile_hook = lambda: holder["h"]
        sys.modules["antenv.axon_hooks"] = mod
        antenv.axon_hooks = mod
        mod.set_axon_ntff_profile_hook(
            _ntff_profile_via_ctypes("/opt/axon/libaxon_pjrt.so")
        )
    except Exception as e:
        print(f"profile shim unavailable: {e}")


def kernel(**inputs) -> np.ndarray:
    from concourse import bass_utils

    if "nc" not in _session:
        _session["nc"] = _build()
    nc = _session["nc"]

    in_maps = _prep_in_maps(inputs)
    trace = os.environ.get("KERNEL_TRACE", "0") == "1"
    if trace:
        _install_profile_shim()
    res = bass_utils.run_bass_kernel_spmd(
        nc, in_maps, core_ids=list(range(N_CORES)), trace=trace
    )
    if trace and res.exec_time_ns is not None:
        print(f"HW exec time: {res.exec_time_ns} ns")
        kernel.last_exec_time_ns = res.exec_time_ns
    kernel.last_results = res

    out = np.zeros(3, np.float64)
    for r in res.results:
        out += r["q"][:, 0].astype(np.float64)
    return out.astype(np.float32)
